# revision 4
# baseline (speedup 1.0000x reference)
"""Causal multi-head attention (B=2, T=2048, D=1024, H=16) on 8 trn2 cores.

Sharding: data-parallel over batch (2) x tensor-parallel over heads (4 groups
of 4 heads). Core c handles batch c//4... (b = c//4, head group c%4).
Each core computes q/k/v projections for its 256 feature columns, causal
attention for its 4 heads, and a partial row-parallel output projection.
Host sums the 4 partials per batch and adds bo.
"""

import sys

if "/opt/trn_rl_repo" not in sys.path:
    sys.path.insert(0, "/opt/trn_rl_repo")

import numpy as np

import concourse.bass as bass
import concourse.mybir as mybir
import concourse.tile as tile
from concourse import bacc

F32 = mybir.dt.float32
F32R = mybir.dt.float32r

B, T, D, H, HD = 2, 2048, 1024, 16, 64
SCALE = float(D) ** -0.5
NCORES = 8
HPC = 4  # heads per core
JS = HPC * HD  # 256 feature columns per core
NT = T // 128  # 16 t-chunks
ND = D // 128  # 8 d-chunks
NG = T // 512  # 4 query groups
MASKVAL = -1e30

_CACHE = {}


def build():
    nc = bacc.Bacc("TRN2", target_bir_lowering=False, num_devices=NCORES)

    x_d = nc.dram_tensor("x", [T, D], F32, kind="ExternalInput")
    wq_d = nc.dram_tensor("wq", [D, JS], F32R, kind="ExternalInput")
    wk_d = nc.dram_tensor("wk", [D, JS], F32R, kind="ExternalInput")
    wv_d = nc.dram_tensor("wv", [D, JS], F32R, kind="ExternalInput")
    bq_d = nc.dram_tensor("bq", [JS], F32, kind="ExternalInput")
    bk_d = nc.dram_tensor("bk", [JS], F32, kind="ExternalInput")
    bv_d = nc.dram_tensor("bv", [JS], F32, kind="ExternalInput")
    wo_d = nc.dram_tensor("wo", [JS, D], F32R, kind="ExternalInput")
    out_d = nc.dram_tensor("out", [T, D], F32, kind="ExternalOutput")

    with tile.TileContext(nc) as tc:
        with (
            tc.tile_pool(name="consts", bufs=1) as consts,
            tc.tile_pool(name="persist", bufs=1) as persist,
        ):
            # --- constants / weights ---
            ident = consts.tile([128, 128], F32)
            nc.gpsimd.memset(ident, 0.0)
            nc.gpsimd.affine_select(
                out=ident, in_=ident, compare_op=mybir.AluOpType.not_equal,
                fill=1.0, base=0, pattern=[[-1, 128]], channel_multiplier=1,
            )
            # causal mask band: M[p, s] = 0 if s >= p + 512 else MASKVAL
            # slice M[:, 512-junk:] gives [p, f]-mask: 0 iff f >= p + junk
            mband = consts.tile([128, 640], F32)
            nc.gpsimd.memset(mband, 0.0)
            nc.gpsimd.affine_select(
                out=mband, in_=mband, compare_op=mybir.AluOpType.is_ge,
                fill=MASKVAL, base=-512, pattern=[[1, 640]], channel_multiplier=-1,
            )

            wq_sb = consts.tile([128, ND, JS], F32R)
            wk_sb = consts.tile([128, ND, JS], F32R)
            wv_sb = consts.tile([128, ND, JS], F32R)
            for w_sb, w_d in ((wq_sb, wq_d), (wk_sb, wk_d), (wv_sb, wv_d)):
                nc.sync.dma_start(
                    out=w_sb, in_=w_d.ap().rearrange("(c p) j -> p c j", p=128)
                )
            wo_sb = consts.tile([128, 2, D], F32R)
            nc.sync.dma_start(
                out=wo_sb, in_=wo_d.ap().rearrange("(c p) n -> p c n", p=128)
            )
            bq_sb = consts.tile([128, 2], F32)
            bk_sb = consts.tile([128, 2], F32)
            nc.sync.dma_start(out=bq_sb, in_=bq_d.ap().rearrange("(c p) -> p c", p=128))
            nc.sync.dma_start(out=bk_sb, in_=bk_d.ap().rearrange("(c p) -> p c", p=128))
            # bv broadcast to all partitions: [128, 256]
            bv_bc = consts.tile([128, JS], F32)
            nc.gpsimd.dma_start(
                out=bv_bc,
                in_=bass.AP(tensor=bv_d, offset=0, ap=[[0, 128], [1, JS]]),
            )
            ones_sb = consts.tile([128, HPC * NT], F32)
            nc.vector.memset(ones_sb, 1.0)

            # --- persistent activations ---
            qT = persist.tile([128, 2, T], F32R)  # [j % 128, j // 128, t]
            kT = persist.tile([128, 2, T], F32R)
            vv = persist.tile([128, HPC, NT, HD + 1], F32R)  # v + ones col
            oT = persist.tile([128, 2, T], F32R)  # attn out^T (normalized)

            # ================= phase 0 + 1: transpose x, project =========
            with (
                tc.tile_pool(name="xstage", bufs=3) as xstage,
                tc.tile_pool(name="xTpool", bufs=1) as xTpool,
                tc.tile_pool(name="psA", bufs=2, space="PSUM") as psA,
                tc.tile_pool(name="psB", bufs=2, space="PSUM") as psB,
            ):
                xT = xTpool.tile([128, ND, T], F32R)  # [d%128, d//128, t]
                for i in range(NT):
                    x_sb = xstage.tile([128, D], F32)
                    nc.sync.dma_start(out=x_sb, in_=x_d.ap()[i * 128:(i + 1) * 128, :])
                    for half in range(2):
                        tp = psA.tile([128, 512], F32)
                        for q4 in range(4):
                            dc = half * 4 + q4
                            nc.tensor.transpose(
                                tp[:, q4 * 128:(q4 + 1) * 128],
                                x_sb[:, dc * 128:(dc + 1) * 128],
                                ident,
                            )
                        dst = xT[:, half * 4:(half + 1) * 4, i * 128:(i + 1) * 128]
                        src = tp.rearrange("p (c f) -> p c f", c=4)
                        if half == 0:
                            nc.vector.tensor_copy(dst, src)
                        else:
                            nc.scalar.copy(dst, src)

                # q^T, k^T: [j, t] with j on partitions (2 chunks)
                for w_sb, b_sb, dstT in ((wq_sb, bq_sb, qT), (wk_sb, bk_sb, kT)):
                    for jc in range(2):
                        for tg in range(NG):
                            ps = psB.tile([128, 512], F32)
                            for dc in range(ND):
                                nc.tensor.matmul(
                                    ps,
                                    w_sb[:, dc, jc * 128:(jc + 1) * 128],
                                    xT[:, dc, tg * 512:(tg + 1) * 512],
                                    start=(dc == 0),
                                    stop=(dc == ND - 1),
                                )
                            nc.vector.tensor_scalar_add(
                                out=dstT[:, jc, tg * 512:(tg + 1) * 512],
                                in0=ps,
                                scalar1=b_sb[:, jc:jc + 1],
                            )
                # v in natural [t, j] layout, per t-chunk
                for i in range(NT):
                    ps = psB.tile([128, 512], F32)
                    for dc in range(ND):
                        nc.tensor.matmul(
                            ps[:, :JS],
                            xT[:, dc, i * 128:(i + 1) * 128],
                            wv_sb[:, dc, :],
                            start=(dc == 0),
                            stop=(dc == ND - 1),
                        )
                    nc.vector.tensor_add(
                        out=vv[:, :, i, 0:HD],
                        in0=ps[:, :JS].rearrange("p (h c) -> p h c", h=HPC),
                        in1=bv_bc.rearrange("p (h c) -> p h c", h=HPC),
                    )
                # ones column for the softmax denominator
                nc.vector.tensor_copy(
                    vv[:, :, :, HD:HD + 1],
                    ones_sb.rearrange("p (h i o) -> p h i o", h=HPC, o=1),
                )

            # ================= phase 2: attention per head ===============
            with (
                tc.tile_pool(name="psS", bufs=1, space="PSUM") as psS,
                tc.tile_pool(name="psO", bufs=4, space="PSUM") as psO,
                tc.tile_pool(name="esb", bufs=3) as esb,
                tc.tile_pool(name="nrm", bufs=4) as nrm,
            ):
                for h in range(HPC):
                    jc, hr = h // 2, (h % 2) * 64
                    accs = [
                        psO.tile([128, 512], F32, tag="oacc", name=f"oacc_{h}_{g}")
                        for g in range(NG)
                    ]
                    for ck in range(NT):
                        g0 = ck // 4
                        ngr = NG - g0
                        w = ngr * 512
                        ps = psS.tile([128, 2048], F32)
                        for gi in range(ngr):
                            g = g0 + gi
                            nc.tensor.matmul(
                                ps[:, gi * 512:(gi + 1) * 512],
                                kT[hr:hr + 64, jc, ck * 128:(ck + 1) * 128],
                                qT[hr:hr + 64, jc, g * 512:(g + 1) * 512],
                                start=True,
                                stop=True,
                            )
                        # additive causal mask on the diagonal-crossing span
                        junk = ck * 128 - g0 * 512
                        nc.vector.tensor_add(
                            out=ps[:, :junk + 128],
                            in0=ps[:, :junk + 128],
                            in1=mband[:, 512 - junk:640],
                        )
                        es = esb.tile([128, 2048], F32R)
                        nc.scalar.activation(
                            es[:, :w],
                            ps[:, :w],
                            mybir.ActivationFunctionType.Exp,
                            scale=SCALE,
                        )
                        for gi in range(ngr):
                            g = g0 + gi
                            nc.tensor.matmul(
                                accs[g][0:HD + 1, :],
                                vv[:, h, ck, :],
                                es[:, gi * 512:(gi + 1) * 512],
                                start=(ck == 0),
                                stop=(ck == 4 * g + 3),
                            )
                    for g in range(NG):
                        rc = nrm.tile([1, 512], F32, tag="rc")
                        nc.vector.reciprocal(rc, accs[g][HD:HD + 1, :])
                        rb = nrm.tile([64, 512], F32, tag="rb")
                        nc.gpsimd.partition_broadcast(rb, rc)
                        nc.vector.tensor_mul(
                            oT[hr:hr + 64, jc, g * 512:(g + 1) * 512],
                            accs[g][0:HD, :],
                            rb,
                        )

            # ================= phase 3: output projection ================
            with (
                tc.tile_pool(name="ps3", bufs=4, space="PSUM") as ps3,
                tc.tile_pool(name="osb", bufs=4) as osb,
            ):
                for i in range(NT):
                    for ng in range(2):
                        ps = ps3.tile([128, 512], F32)
                        for jc in range(2):
                            nc.tensor.matmul(
                                ps,
                                oT[:, jc, i * 128:(i + 1) * 128],
                                wo_sb[:, jc, ng * 512:(ng + 1) * 512],
                                start=(jc == 0),
                                stop=(jc == 1),
                            )
                        ob = osb.tile([128, 512], F32)
                        if (i + ng) % 2 == 0:
                            nc.vector.tensor_copy(ob, ps)
                        else:
                            nc.scalar.copy(ob, ps)
                        nc.sync.dma_start(
                            out=out_d.ap()[
                                i * 128:(i + 1) * 128, ng * 512:(ng + 1) * 512
                            ],
                            in_=ob,
                        )

    nc.compile()
    return nc


def _in_maps(inputs):
    x = np.ascontiguousarray(np.asarray(inputs["x"], dtype=np.float32))
    maps = []
    for c in range(NCORES):
        b, g = c // HPC, c % HPC
        js = slice(g * JS, (g + 1) * JS)
        maps.append(
            {
                "x": np.ascontiguousarray(x[b]),
                "wq": np.ascontiguousarray(np.asarray(inputs["wq"], np.float32)[:, js]),
                "wk": np.ascontiguousarray(np.asarray(inputs["wk"], np.float32)[:, js]),
                "wv": np.ascontiguousarray(np.asarray(inputs["wv"], np.float32)[:, js]),
                "bq": np.ascontiguousarray(np.asarray(inputs["bq"], np.float32)[js]),
                "bk": np.ascontiguousarray(np.asarray(inputs["bk"], np.float32)[js]),
                "bv": np.ascontiguousarray(np.asarray(inputs["bv"], np.float32)[js]),
                "wo": np.ascontiguousarray(np.asarray(inputs["wo"], np.float32)[js, :]),
            }
        )
    return maps


def kernel(**inputs) -> np.ndarray:
    from concourse.bass_utils import run_bass_kernel_spmd

    if "nc" not in _CACHE:
        _CACHE["nc"] = build()
    nc = _CACHE["nc"]
    maps = _in_maps(inputs)
    res = run_bass_kernel_spmd(nc, maps, core_ids=list(range(NCORES)))
    out = np.zeros((B, T, D), dtype=np.float32)
    for c in range(NCORES):
        out[c // HPC] += res.results[c]["out"]
    out += np.asarray(inputs["bo"], np.float32)[None, None, :]
    return out


if __name__ == "__main__":
    rng = np.random.default_rng(0)
    s = D ** -0.5
    inputs = {
        "x": rng.standard_normal((B, T, D)).astype(np.float32),
        "wq": (rng.standard_normal((D, D)) * s).astype(np.float32),
        "bq": (rng.standard_normal(D) * 0.01).astype(np.float32),
        "wk": (rng.standard_normal((D, D)) * s).astype(np.float32),
        "bk": (rng.standard_normal(D) * 0.01).astype(np.float32),
        "wv": (rng.standard_normal((D, D)) * s).astype(np.float32),
        "bv": (rng.standard_normal(D) * 0.01).astype(np.float32),
        "wo": (rng.standard_normal((D, D)) * s).astype(np.float32),
        "bo": (rng.standard_normal(D) * 0.01).astype(np.float32),
    }
    got = kernel(**inputs)
    # numpy reference
    x = inputs["x"].astype(np.float64)
    q = (x @ inputs["wq"] + inputs["bq"]).reshape(B, T, H, HD).transpose(0, 2, 1, 3)
    k = (x @ inputs["wk"] + inputs["bk"]).reshape(B, T, H, HD).transpose(0, 2, 1, 3)
    v = (x @ inputs["wv"] + inputs["bv"]).reshape(B, T, H, HD).transpose(0, 2, 1, 3)
    sc = np.einsum("bhqd,bhkd->bhqk", q, k) * SCALE
    mask = np.tril(np.ones((T, T), bool))
    sc = np.where(mask[None, None], sc, -np.inf)
    sc = sc - sc.max(-1, keepdims=True)
    p = np.exp(sc)
    p /= p.sum(-1, keepdims=True)
    o = np.einsum("bhqk,bhkd->bhqd", p, v).transpose(0, 2, 1, 3).reshape(B, T, D)
    exp = o @ inputs["wo"] + inputs["bo"]
    rel = np.linalg.norm(got - exp) / np.linalg.norm(exp)
    print(f"Relative error: {rel:.3e}")


# revision 5
# speedup vs baseline: 14.5584x; 14.5584x over previous
"""Causal multi-head attention (B=2, T=2048, D=1024, H=16) on 8 trn2 cores.

Sharding: data-parallel over batch (2) x tensor-parallel over heads (4 groups
of 4 heads): core c handles batch c//4, head group c%4. Each core computes
q/k/v projections for its 256 feature columns, causal attention for its 4
heads, and a partial row-parallel output projection. The host sums the 4
partials per batch and adds bo.
"""

import sys

if "/opt/trn_rl_repo" not in sys.path:
    sys.path.insert(0, "/opt/trn_rl_repo")

import numpy as np

import concourse.bass as bass
import concourse.mybir as mybir
import concourse.tile as tile
from concourse import bacc

F32 = mybir.dt.float32
F32R = mybir.dt.float32r
EXP = mybir.ActivationFunctionType.Exp

B, T, D, H, HD = 2, 2048, 1024, 16, 64
SCALE = float(D) ** -0.5
NCORES = 8
HPC = 4  # heads per core
JS = HPC * HD  # 256 feature columns per core
NT = T // 128  # 16 t-chunks
ND = D // 128  # 8 d-chunks
NG = T // 512  # 4 query groups
MASKVAL = -1e30

_CACHE = {}


def _emit_consts(nc, consts, dram):
    c = {}
    c["ident"] = consts.tile([128, 128], F32, name="ident")
    nc.gpsimd.memset(c["ident"], 0.0)
    nc.gpsimd.affine_select(
        out=c["ident"], in_=c["ident"], compare_op=mybir.AluOpType.not_equal,
        fill=1.0, base=0, pattern=[[-1, 128]], channel_multiplier=1,
    )
    # causal mask band: M[p, s] = 0 if s >= p + 512 else MASKVAL
    # slice M[:, 512-junk:] is a [p, f]-mask: 0 iff f >= p + junk
    c["mband"] = consts.tile([128, 640], F32, name="mband")
    nc.gpsimd.memset(c["mband"], 0.0)
    nc.gpsimd.affine_select(
        out=c["mband"], in_=c["mband"], compare_op=mybir.AluOpType.is_ge,
        fill=MASKVAL, base=-512, pattern=[[1, 640]], channel_multiplier=-1,
    )
    c["wq"] = consts.tile([128, ND, JS], F32R, name="wq_sb")
    c["wk"] = consts.tile([128, ND, JS], F32R, name="wk_sb")
    c["wv"] = consts.tile([128, ND, JS], F32R, name="wv_sb")
    for key in ("wq", "wk", "wv"):
        nc.sync.dma_start(
            out=c[key], in_=dram[key].ap().rearrange("(c p) j -> p c j", p=128)
        )
    c["wo"] = consts.tile([128, 2, D], F32R, name="wo_sb")
    nc.sync.dma_start(
        out=c["wo"], in_=dram["wo"].ap().rearrange("(c p) n -> p c n", p=128)
    )
    c["bq"] = consts.tile([128, 2], F32, name="bq_sb")
    c["bk"] = consts.tile([128, 2], F32, name="bk_sb")
    nc.sync.dma_start(out=c["bq"], in_=dram["bq"].ap().rearrange("(c p) -> p c", p=128))
    nc.sync.dma_start(out=c["bk"], in_=dram["bk"].ap().rearrange("(c p) -> p c", p=128))
    c["bv"] = consts.tile([128, JS], F32, name="bv_bc")
    nc.gpsimd.dma_start(
        out=c["bv"], in_=bass.AP(tensor=dram["bv"], offset=0, ap=[[0, 128], [1, JS]])
    )
    c["ones"] = consts.tile([128, HPC * NT], F32, name="ones_sb")
    nc.vector.memset(c["ones"], 1.0)
    return c


def _emit_body(nc, tc, c, persist, dram, rep):
    """One full attention pass (phases 0-3)."""
    r = f"r{rep}"
    qT = persist["qT"]
    kT = persist["kT"]
    vv = persist["vv"]
    oT = persist["oT"]
    x_d = dram["x"]
    out_d = dram["out"]

    # ---------------- phase 0 + 1: transpose x, project -----------------
    with (
        tc.tile_pool(name=f"xstage{r}", bufs=3) as xstage,
        tc.tile_pool(name=f"xTpool{r}", bufs=1) as xTpool,
        tc.tile_pool(name=f"psA{r}", bufs=2, space="PSUM") as psA,
        tc.tile_pool(name=f"psB{r}", bufs=2, space="PSUM") as psB,
    ):
        xT = xTpool.tile([128, ND, T], F32R, name=f"xT{r}")
        for i in range(NT):
            x_sb = xstage.tile([128, D], F32, name=f"x_sb{r}", tag="x_sb")
            nc.sync.dma_start(out=x_sb, in_=x_d.ap()[i * 128:(i + 1) * 128, :])
            for half in range(2):
                tp = psA.tile([128, 512], F32, name=f"tp{r}", tag="tp")
                for q4 in range(4):
                    dc = half * 4 + q4
                    nc.tensor.transpose(
                        tp[:, q4 * 128:(q4 + 1) * 128],
                        x_sb[:, dc * 128:(dc + 1) * 128],
                        c["ident"],
                    )
                dst = xT[:, half * 4:(half + 1) * 4, i * 128:(i + 1) * 128]
                src = tp.rearrange("p (c f) -> p c f", c=4)
                if half == 0:
                    nc.vector.tensor_copy(dst, src)
                else:
                    nc.scalar.copy(dst, src)

        # q^T, k^T: [j, t] layouts with j on partitions (2 chunks)
        for w_sb, b_sb, dstT in ((c["wq"], c["bq"], qT), (c["wk"], c["bk"], kT)):
            for jc in range(2):
                for tg in range(NG):
                    ps = psB.tile([128, 512], F32, name=f"psqk{r}", tag="psb")
                    for dc in range(ND):
                        nc.tensor.matmul(
                            ps,
                            w_sb[:, dc, jc * 128:(jc + 1) * 128],
                            xT[:, dc, tg * 512:(tg + 1) * 512],
                            start=(dc == 0),
                            stop=(dc == ND - 1),
                        )
                    nc.vector.tensor_scalar_add(
                        out=dstT[:, jc, tg * 512:(tg + 1) * 512],
                        in0=ps,
                        scalar1=b_sb[:, jc:jc + 1],
                    )
        # v in natural [t, j] layout, per t-chunk
        for i in range(NT):
            ps = psB.tile([128, 512], F32, name=f"psv{r}", tag="psb")
            for dc in range(ND):
                nc.tensor.matmul(
                    ps[:, :JS],
                    xT[:, dc, i * 128:(i + 1) * 128],
                    c["wv"][:, dc, :],
                    start=(dc == 0),
                    stop=(dc == ND - 1),
                )
            nc.vector.tensor_add(
                out=vv[:, :, i, 0:HD],
                in0=ps[:, :JS].rearrange("p (h c) -> p h c", h=HPC),
                in1=c["bv"].rearrange("p (h c) -> p h c", h=HPC),
            )
        nc.vector.tensor_copy(
            vv[:, :, :, HD:HD + 1],
            c["ones"].rearrange("p (h i o) -> p h i o", h=HPC, o=1),
        )

    # ---------------- phase 2: attention per head ------------------------
    with (
        tc.tile_pool(name=f"psS{r}", bufs=1, space="PSUM") as psS,
        tc.tile_pool(name=f"psO{r}", bufs=4, space="PSUM") as psO,
        tc.tile_pool(name=f"esb{r}", bufs=3) as esb,
        tc.tile_pool(name=f"nrm{r}", bufs=4) as nrm,
    ):
        for h in range(HPC):
            jc, hr = h // 2, (h % 2) * 64
            accs = [
                psO.tile([128, 512], F32, tag="oacc", name=f"oacc{r}_{h}_{g}")
                for g in range(NG)
            ]
            for ck in range(NT):
                g0 = ck // 4
                ngr = NG - g0
                w = ngr * 512
                ps = psS.tile([128, 2048], F32, name=f"psrow{r}", tag="psrow")
                for gi in range(ngr):
                    g = g0 + gi
                    nc.tensor.matmul(
                        ps[:, gi * 512:(gi + 1) * 512],
                        kT[hr:hr + 64, jc, ck * 128:(ck + 1) * 128],
                        qT[hr:hr + 64, jc, g * 512:(g + 1) * 512],
                        start=True,
                        stop=True,
                    )
                # additive causal mask on the diagonal-crossing span
                junk = ck * 128 - g0 * 512
                nc.vector.tensor_add(
                    out=ps[:, :junk + 128],
                    in0=ps[:, :junk + 128],
                    in1=c["mband"][:, 512 - junk:640],
                )
                es = esb.tile([128, 2048], F32R, name=f"es{r}", tag="es")
                nc.scalar.activation(es[:, :w], ps[:, :w], EXP, scale=SCALE)
                for gi in range(ngr):
                    g = g0 + gi
                    nc.tensor.matmul(
                        accs[g][0:HD + 1, :],
                        vv[:, h, ck, :],
                        es[:, gi * 512:(gi + 1) * 512],
                        start=(ck == 0),
                        stop=(ck == 4 * g + 3),
                    )
            for g in range(NG):
                rc = nrm.tile([1, 512], F32, tag="rc", name=f"rc{r}_{h}_{g}")
                nc.vector.reciprocal(rc, accs[g][HD:HD + 1, :])
                rb = nrm.tile([64, 512], F32, tag="rb", name=f"rb{r}_{h}_{g}")
                nc.gpsimd.partition_broadcast(rb, rc)
                nc.vector.tensor_mul(
                    oT[hr:hr + 64, jc, g * 512:(g + 1) * 512],
                    accs[g][0:HD, :],
                    rb,
                )

    # ---------------- phase 3: output projection -------------------------
    with (
        tc.tile_pool(name=f"ps3{r}", bufs=4, space="PSUM") as ps3,
        tc.tile_pool(name=f"osb{r}", bufs=4) as osb,
    ):
        for i in range(NT):
            for ng in range(2):
                ps = ps3.tile([128, 512], F32, name=f"ps3t{r}", tag="ps3t")
                for jc in range(2):
                    nc.tensor.matmul(
                        ps,
                        oT[:, jc, i * 128:(i + 1) * 128],
                        c["wo"][:, jc, ng * 512:(ng + 1) * 512],
                        start=(jc == 0),
                        stop=(jc == 1),
                    )
                ob = osb.tile([128, 512], F32, name=f"ob{r}", tag="ob")
                if (i + ng) % 2 == 0:
                    nc.vector.tensor_copy(ob, ps)
                else:
                    nc.scalar.copy(ob, ps)
                nc.sync.dma_start(
                    out=out_d.ap()[i * 128:(i + 1) * 128, ng * 512:(ng + 1) * 512],
                    in_=ob,
                )


def build(reps=1):
    nc = bacc.Bacc("TRN2", target_bir_lowering=False, num_devices=NCORES)
    dram = {
        "x": nc.dram_tensor("x", [T, D], F32, kind="ExternalInput"),
        "wq": nc.dram_tensor("wq", [D, JS], F32R, kind="ExternalInput"),
        "wk": nc.dram_tensor("wk", [D, JS], F32R, kind="ExternalInput"),
        "wv": nc.dram_tensor("wv", [D, JS], F32R, kind="ExternalInput"),
        "bq": nc.dram_tensor("bq", [JS], F32, kind="ExternalInput"),
        "bk": nc.dram_tensor("bk", [JS], F32, kind="ExternalInput"),
        "bv": nc.dram_tensor("bv", [JS], F32, kind="ExternalInput"),
        "wo": nc.dram_tensor("wo", [JS, D], F32R, kind="ExternalInput"),
        "out": nc.dram_tensor("out", [T, D], F32, kind="ExternalOutput"),
    }
    with tile.TileContext(nc) as tc:
        with (
            tc.tile_pool(name="consts", bufs=1) as consts,
            tc.tile_pool(name="persist", bufs=1) as persist_pool,
        ):
            c = _emit_consts(nc, consts, dram)
            persist = {
                "qT": persist_pool.tile([128, 2, T], F32R, name="qT"),
                "kT": persist_pool.tile([128, 2, T], F32R, name="kT"),
                "vv": persist_pool.tile([128, HPC, NT, HD + 1], F32R, name="vv"),
                "oT": persist_pool.tile([128, 2, T], F32R, name="oT"),
            }
            for rep in range(reps):
                _emit_body(nc, tc, c, persist, dram, rep)
    nc.compile()
    return nc


def _in_maps(inputs):
    x = np.ascontiguousarray(np.asarray(inputs["x"], dtype=np.float32))
    maps = []
    for cc in range(NCORES):
        b, g = cc // HPC, cc % HPC
        js = slice(g * JS, (g + 1) * JS)
        maps.append(
            {
                "x": np.ascontiguousarray(x[b]),
                "wq": np.ascontiguousarray(np.asarray(inputs["wq"], np.float32)[:, js]),
                "wk": np.ascontiguousarray(np.asarray(inputs["wk"], np.float32)[:, js]),
                "wv": np.ascontiguousarray(np.asarray(inputs["wv"], np.float32)[:, js]),
                "bq": np.ascontiguousarray(np.asarray(inputs["bq"], np.float32)[js]),
                "bk": np.ascontiguousarray(np.asarray(inputs["bk"], np.float32)[js]),
                "bv": np.ascontiguousarray(np.asarray(inputs["bv"], np.float32)[js]),
                "wo": np.ascontiguousarray(np.asarray(inputs["wo"], np.float32)[js, :]),
            }
        )
    return maps


def kernel(**inputs) -> np.ndarray:
    from concourse.bass_utils import run_bass_kernel_spmd

    if "nc" not in _CACHE:
        _CACHE["nc"] = build()
    nc = _CACHE["nc"]
    maps = _in_maps(inputs)
    res = run_bass_kernel_spmd(nc, maps, core_ids=list(range(NCORES)))
    out = np.zeros((B, T, D), dtype=np.float32)
    for cc in range(NCORES):
        out[cc // HPC] += res.results[cc]["out"]
    out += np.asarray(inputs["bo"], np.float32)[None, None, :]
    return out


# revision 7
# speedup vs baseline: 37.4548x; 2.5727x over previous
"""Causal multi-head attention (B=2, T=2048, D=1024, H=16) on 8 trn2 cores.

Sharding: data-parallel over batch (2) x tensor-parallel over heads (4 groups
of 4 heads): core c handles batch c//4, head group c%4. Each core computes
q/k/v projections for its 256 feature columns, causal attention for its 4
heads, and a partial row-parallel output projection. The host sums the 4
partials per batch and adds bo.
"""

import sys

if "/opt/trn_rl_repo" not in sys.path:
    sys.path.insert(0, "/opt/trn_rl_repo")

import numpy as np

import concourse.bass as bass
import concourse.mybir as mybir
import concourse.tile as tile
from concourse import bacc

F32 = mybir.dt.float32
F32R = mybir.dt.float32r
EXP = mybir.ActivationFunctionType.Exp

B, T, D, H, HD = 2, 2048, 1024, 16, 64
SCALE = float(D) ** -0.5
NCORES = 8
HPC = 4  # heads per core
JS = HPC * HD  # 256 feature columns per core
NT = T // 128  # 16 t-chunks
ND = D // 128  # 8 d-chunks
NG = T // 512  # 4 query groups
MASKVAL = -1e30

_CACHE = {}


def _emit_consts(nc, consts, dram):
    c = {}
    c["ident"] = consts.tile([128, 128], F32, name="ident")
    nc.gpsimd.memset(c["ident"], 0.0)
    nc.gpsimd.affine_select(
        out=c["ident"], in_=c["ident"], compare_op=mybir.AluOpType.not_equal,
        fill=1.0, base=0, pattern=[[-1, 128]], channel_multiplier=1,
    )
    # causal mask band: M[p, s] = 0 if s >= p + 512 else MASKVAL
    # slice M[:, 512-junk:] is a [p, f]-mask: 0 iff f >= p + junk
    c["mband"] = consts.tile([128, 640], F32, name="mband")
    nc.gpsimd.memset(c["mband"], 0.0)
    nc.gpsimd.affine_select(
        out=c["mband"], in_=c["mband"], compare_op=mybir.AluOpType.is_ge,
        fill=MASKVAL, base=-512, pattern=[[1, 640]], channel_multiplier=-1,
    )
    c["wq"] = consts.tile([128, ND, JS], F32R, name="wq_sb")
    c["wk"] = consts.tile([128, ND, JS], F32R, name="wk_sb")
    c["wv"] = consts.tile([128, ND, JS], F32R, name="wv_sb")
    for key in ("wq", "wk", "wv"):
        nc.sync.dma_start(
            out=c[key], in_=dram[key].ap().rearrange("(c p) j -> p c j", p=128)
        )
    c["wo"] = consts.tile([128, 2, D], F32R, name="wo_sb")
    nc.sync.dma_start(
        out=c["wo"], in_=dram["wo"].ap().rearrange("(c p) n -> p c n", p=128)
    )
    c["bq"] = consts.tile([128, 2], F32, name="bq_sb")
    c["bk"] = consts.tile([128, 2], F32, name="bk_sb")
    nc.sync.dma_start(out=c["bq"], in_=dram["bq"].ap().rearrange("(c p) -> p c", p=128))
    nc.sync.dma_start(out=c["bk"], in_=dram["bk"].ap().rearrange("(c p) -> p c", p=128))
    c["bv"] = consts.tile([128, JS], F32, name="bv_bc")
    nc.gpsimd.dma_start(
        out=c["bv"], in_=bass.AP(tensor=dram["bv"], offset=0, ap=[[0, 128], [1, JS]])
    )
    c["ones"] = consts.tile([128, HPC * NT], F32, name="ones_sb")
    nc.vector.memset(c["ones"], 1.0)
    return c


def _emit_body(nc, tc, c, persist, dram, rep):
    """One full attention pass (phases 0-3)."""
    r = f"r{rep}"
    qT = persist["qT"]
    kT = persist["kT"]
    vv = persist["vv"]
    oT = persist["oT"]
    x_d = dram["x"]
    out_d = dram["out"]

    # ---------------- phase 0 + 1: transpose x, project -----------------
    with (
        tc.tile_pool(name=f"xstage{r}", bufs=3) as xstage,
        tc.tile_pool(name=f"xTpool{r}", bufs=1) as xTpool,
        tc.tile_pool(name=f"psA{r}", bufs=2, space="PSUM") as psA,
        tc.tile_pool(name=f"psB{r}", bufs=2, space="PSUM") as psB,
    ):
        xT = xTpool.tile([128, ND, T], F32R, name=f"xT{r}")
        for i in range(NT):
            x_sb = xstage.tile([128, D], F32, name=f"x_sb{r}", tag="x_sb")
            nc.sync.dma_start(out=x_sb, in_=x_d.ap()[i * 128:(i + 1) * 128, :])
            for half in range(2):
                tp = psA.tile([128, 512], F32, name=f"tp{r}", tag="tp")
                for q4 in range(4):
                    dc = half * 4 + q4
                    nc.tensor.transpose(
                        tp[:, q4 * 128:(q4 + 1) * 128],
                        x_sb[:, dc * 128:(dc + 1) * 128],
                        c["ident"],
                    )
                dst = xT[:, half * 4:(half + 1) * 4, i * 128:(i + 1) * 128]
                src = tp.rearrange("p (c f) -> p c f", c=4)
                if half == 0:
                    nc.vector.tensor_copy(dst, src)
                else:
                    nc.scalar.copy(dst, src)

        # q^T, k^T: [j, t] layouts with j on partitions (2 chunks)
        for w_sb, b_sb, dstT in ((c["wq"], c["bq"], qT), (c["wk"], c["bk"], kT)):
            for jc in range(2):
                for tg in range(NG):
                    ps = psB.tile([128, 512], F32, name=f"psqk{r}", tag="psb")
                    for dc in range(ND):
                        nc.tensor.matmul(
                            ps,
                            w_sb[:, dc, jc * 128:(jc + 1) * 128],
                            xT[:, dc, tg * 512:(tg + 1) * 512],
                            start=(dc == 0),
                            stop=(dc == ND - 1),
                        )
                    nc.vector.tensor_scalar_add(
                        out=dstT[:, jc, tg * 512:(tg + 1) * 512],
                        in0=ps,
                        scalar1=b_sb[:, jc:jc + 1],
                    )
        # v in natural [t, j] layout, per t-chunk
        for i in range(NT):
            ps = psB.tile([128, 512], F32, name=f"psv{r}", tag="psb")
            for dc in range(ND):
                nc.tensor.matmul(
                    ps[:, :JS],
                    xT[:, dc, i * 128:(i + 1) * 128],
                    c["wv"][:, dc, :],
                    start=(dc == 0),
                    stop=(dc == ND - 1),
                )
            nc.vector.tensor_add(
                out=vv[:, :, i, 0:HD],
                in0=ps[:, :JS].rearrange("p (h c) -> p h c", h=HPC),
                in1=c["bv"].rearrange("p (h c) -> p h c", h=HPC),
            )
        nc.vector.tensor_copy(
            vv[:, :, :, HD:HD + 1],
            c["ones"].rearrange("p (h i o) -> p h i o", h=HPC, o=1),
        )

    # ---------------- phase 2: attention per head ------------------------
    with (
        tc.tile_pool(name=f"psS{r}", bufs=2, space="PSUM") as psS,
        tc.tile_pool(name=f"psO{r}", bufs=4, space="PSUM") as psO,
        tc.tile_pool(name=f"esb{r}", bufs=4) as esb,
        tc.tile_pool(name=f"nrm{r}", bufs=4) as nrm,
    ):
        for h in range(HPC):
            jc, hr = h // 2, (h % 2) * 64
            accs = [
                psO.tile([128, 512], F32, tag="oacc", name=f"oacc{r}_{h}_{g}")
                for g in range(NG)
            ]
            for ck in range(NT):
                g0 = ck // 4
                ngr = NG - g0
                junk = ck * 128 - g0 * 512
                # process groups in 2-group (2-bank) pieces, double-buffered
                for pg in range(g0, NG, 2):
                    pn = min(2, NG - pg)
                    ps = psS.tile([128, 1024], F32, name=f"psrow{r}", tag="psrow")
                    for gi in range(pn):
                        g = pg + gi
                        nc.tensor.matmul(
                            ps[:, gi * 512:(gi + 1) * 512],
                            kT[hr:hr + 64, jc, ck * 128:(ck + 1) * 128],
                            qT[hr:hr + 64, jc, g * 512:(g + 1) * 512],
                            start=True,
                            stop=True,
                        )
                    if pg == g0:
                        # additive causal mask on the diagonal-crossing span
                        nc.vector.tensor_add(
                            out=ps[:, :junk + 128],
                            in0=ps[:, :junk + 128],
                            in1=c["mband"][:, 512 - junk:640],
                        )
                    es = esb.tile([128, 1024], F32R, name=f"es{r}", tag="es")
                    nc.scalar.activation(
                        es[:, :pn * 512], ps[:, :pn * 512], EXP, scale=SCALE
                    )
                    for gi in range(pn):
                        g = pg + gi
                        nc.tensor.matmul(
                            accs[g][0:HD + 1, :],
                            vv[:, h, ck, :],
                            es[:, gi * 512:(gi + 1) * 512],
                            start=(ck == 0),
                            stop=(ck == 4 * g + 3),
                        )
            for g in range(NG):
                rc = nrm.tile([1, 512], F32, tag="rc", name=f"rc{r}_{h}_{g}")
                nc.vector.reciprocal(rc, accs[g][HD:HD + 1, :])
                rb = nrm.tile([64, 512], F32, tag="rb", name=f"rb{r}_{h}_{g}")
                nc.gpsimd.partition_broadcast(rb, rc)
                nc.vector.tensor_mul(
                    oT[hr:hr + 64, jc, g * 512:(g + 1) * 512],
                    accs[g][0:HD, :],
                    rb,
                )

    # ---------------- phase 3: output projection -------------------------
    with (
        tc.tile_pool(name=f"ps3{r}", bufs=4, space="PSUM") as ps3,
        tc.tile_pool(name=f"osb{r}", bufs=4) as osb,
    ):
        for i in range(NT):
            for ng in range(2):
                ps = ps3.tile([128, 512], F32, name=f"ps3t{r}", tag="ps3t")
                for jc in range(2):
                    nc.tensor.matmul(
                        ps,
                        oT[:, jc, i * 128:(i + 1) * 128],
                        c["wo"][:, jc, ng * 512:(ng + 1) * 512],
                        start=(jc == 0),
                        stop=(jc == 1),
                    )
                ob = osb.tile([128, 512], F32, name=f"ob{r}", tag="ob")
                if (i + ng) % 2 == 0:
                    nc.vector.tensor_copy(ob, ps)
                else:
                    nc.scalar.copy(ob, ps)
                nc.sync.dma_start(
                    out=out_d.ap()[i * 128:(i + 1) * 128, ng * 512:(ng + 1) * 512],
                    in_=ob,
                )


def build(reps=1):
    nc = bacc.Bacc("TRN2", target_bir_lowering=False, num_devices=NCORES)
    dram = {
        "x": nc.dram_tensor("x", [T, D], F32, kind="ExternalInput"),
        "wq": nc.dram_tensor("wq", [D, JS], F32R, kind="ExternalInput"),
        "wk": nc.dram_tensor("wk", [D, JS], F32R, kind="ExternalInput"),
        "wv": nc.dram_tensor("wv", [D, JS], F32R, kind="ExternalInput"),
        "bq": nc.dram_tensor("bq", [JS], F32, kind="ExternalInput"),
        "bk": nc.dram_tensor("bk", [JS], F32, kind="ExternalInput"),
        "bv": nc.dram_tensor("bv", [JS], F32, kind="ExternalInput"),
        "wo": nc.dram_tensor("wo", [JS, D], F32R, kind="ExternalInput"),
        "out": nc.dram_tensor("out", [T, D], F32, kind="ExternalOutput"),
    }
    with tile.TileContext(nc) as tc:
        with (
            tc.tile_pool(name="consts", bufs=1) as consts,
            tc.tile_pool(name="persist", bufs=1) as persist_pool,
        ):
            c = _emit_consts(nc, consts, dram)
            persist = {
                "qT": persist_pool.tile([128, 2, T], F32R, name="qT"),
                "kT": persist_pool.tile([128, 2, T], F32R, name="kT"),
                "vv": persist_pool.tile([128, HPC, NT, HD + 1], F32R, name="vv"),
                "oT": persist_pool.tile([128, 2, T], F32R, name="oT"),
            }
            for rep in range(reps):
                _emit_body(nc, tc, c, persist, dram, rep)
    nc.compile()
    return nc


def _in_maps(inputs):
    x = np.ascontiguousarray(np.asarray(inputs["x"], dtype=np.float32))
    maps = []
    for cc in range(NCORES):
        b, g = cc // HPC, cc % HPC
        js = slice(g * JS, (g + 1) * JS)
        maps.append(
            {
                "x": np.ascontiguousarray(x[b]),
                "wq": np.ascontiguousarray(np.asarray(inputs["wq"], np.float32)[:, js]),
                "wk": np.ascontiguousarray(np.asarray(inputs["wk"], np.float32)[:, js]),
                "wv": np.ascontiguousarray(np.asarray(inputs["wv"], np.float32)[:, js]),
                "bq": np.ascontiguousarray(np.asarray(inputs["bq"], np.float32)[js]),
                "bk": np.ascontiguousarray(np.asarray(inputs["bk"], np.float32)[js]),
                "bv": np.ascontiguousarray(np.asarray(inputs["bv"], np.float32)[js]),
                "wo": np.ascontiguousarray(np.asarray(inputs["wo"], np.float32)[js, :]),
            }
        )
    return maps


def kernel(**inputs) -> np.ndarray:
    from concourse.bass_utils import run_bass_kernel_spmd

    if "nc" not in _CACHE:
        _CACHE["nc"] = build()
    nc = _CACHE["nc"]
    maps = _in_maps(inputs)
    res = run_bass_kernel_spmd(nc, maps, core_ids=list(range(NCORES)))
    out = np.zeros((B, T, D), dtype=np.float32)
    for cc in range(NCORES):
        out[cc // HPC] += res.results[cc]["out"]
    out += np.asarray(inputs["bo"], np.float32)[None, None, :]
    return out
